# revision 16
# baseline (speedup 1.0000x reference)
"""Trainium2 Bass kernel for the diffusion-sampler importance-weight problem.

Math (per batch element b, per z-dim p), derived from the reference:
  z_0 = sigma0 * eps0
  per step t (beta_f = beta[t], beta_b = roll(beta,1)[t]):
    hid   = relu(W1z^T z + c1 + te_t)        c1 = ctx @ W1[Z:] + b1 (host)
    u'    = dt * W2^T hid
    z'    = a_t z + u' + eps'_t              a_t = 1 + beta_f dt,
                                             eps' = bf16(s_t eps), s=sqrt(2 b dt) s0
    logw += 0.5 (eps'/s_t)^2 - k_t (z - c_t z')^2 + log(s_t/sb_t)
            c_t = 1 - beta_b dt, sb = sqrt(2 beta_b dt) s0, k = 0.5/sb^2
  terminal: logw += -0.5 (z_T - mu)^2 + 0.5 eps0^2 + log(sig0)
  output = sum_z mean_b logw

Device computes ONLY the trajectory-dependent sums per (p, t, halfchunk):
  slots[p, 2t+j]  = sum_{b in chunk j} (z_prev - c_t z_next)^2
  slots[p, 64+j]  = sum_{b in chunk j} (z_T - mu)^2
Host adds the noise/constant terms and the k_t weighting.

Device layout: feature-major [Z=128 partitions, batch free], batch sharded
8 ways (BS=2048/core, NP=2 chunks of 1024). Matmuls fp8 DoubleRow (K=256):
  hp  = [W1z; I]^T [z; c1]     (c1-add fused via identity rows)
  zps = a_t z (scaled-ident bf16 MM, variant "pe") + dt*W2^T hs + I^T eps'
W1 matmuls are emitted h-major across the two batch chunks so each DR weight
is loaded once per step.  State z is bf16; zc1 fp8 feeds the next W1.
"""
import math
import numpy as np

B, Z, H, HID, T = 16384, 128, 512, 512, 32
NCORES = 8
BS = B // NCORES          # 2048 batch rows per core
NP = 2                    # batch chunks of 1024 per core
CH = BS // NP             # 1024

_cache: dict = {}
# tuning knobs
KNOB = dict(
    variant="stt",               # pe: z' accumulated in PSUM via a_t-ident MM
                                 # stt: z' via DVE STT (baseline-style)
    relu_dve={(0, 3)},           # (np, h) relu chunks on DVE; rest ACT
    zc1_eng="act",               # fp8 z for next W1: act | dve   (variant pe)
    zc1_halves=True,             # emit zc1 evac as 2x512 halves
    zout_eng="dve",              # bf16 state evac (variant pe): dve | act
    cast_eng="pool",             # variant stt: fp8 cast engine pool|act|dve
    slots="sqdiff",              # sqdiff: fused custom-DVE sum((zin-c*zout)^2)
                                 # wvsq: w=TS, v=TT, sq per sq_eng
    w_eng="dve",                 # w = -c*zout:  pool | dve
    v_eng="dve",                 # v = w + zin:  dve | pool
    sq_eng="act",                # act (Square accum) | dve (TTR custom)
    hmajor=False,                # W1 MMs h-major across chunks (shared LDW)
    eps_bufs=3, hs_bufs=2, psh_bufs=3, psz_bufs=1,
    hi_zc1=True,                 # high_priority on zc1/cast evac
)


def _install_sqdiff():
    """Register a fused custom-DVE op: out = (in0*c0 + in1)^2, accum = sum.

    Computes sum_b (zin - c*zout)^2 in one DVE pass (replaces TS+TT+Square).
    The uop tables are written per-NEFF, so registration here is all that's
    needed; the sha pin is derived by a trial compile.
    """
    import operator
    import re

    from concourse import dve_ops
    from concourse.dve_spec import C0, Spec, Src0, Src1, Zero, sq

    for op in dve_ops.OPS:
        if op.name == "SQDIFF_ACC_ANT":
            return op
    def _ref(in0, in1, c0, c1, c2):
        b = (in0.astype(np.float32) * c0 + in1.astype(np.float32)) ** 2
        return b, b.reshape(b.shape[0], -1).sum(axis=-1, keepdims=True)

    spec = Spec(body=sq(Src0 * C0 + Src1), accum=operator.add,
                accum_init=Zero, reference=_ref)
    op = dve_ops.DveOp("SQDIFF_ACC_ANT", spec, subdim=False, uops_sha={})
    dve_ops._SUB_OPCODE_FOR_NAME[op.name] = (
        max(dve_ops._SUB_OPCODE_FOR_NAME.values()) + 1)
    for ver in ("v3", "v4"):
        try:
            dve_ops._COMPILE_CACHE.pop((op.name, ver), None)
            op.compile(ver)
        except ValueError as e:
            m = re.search(r'\["{}"\]="([0-9a-f]+)"'.format(ver), str(e))
            assert m, f"could not pin sha for {ver}: {e}"
            op.uops_sha[ver] = m.group(1)
            dve_ops._COMPILE_CACHE.pop((op.name, ver), None)
    dve_ops.OPS.append(op)
    dve_ops.CUSTOM_DVE_SPECS[op.name] = spec
    return op


def _default_consts():
    """c_t from the deterministic cosine beta schedule (matches setup_inputs)."""
    ts = np.linspace(1.0, 0.0, T)
    beta = (1.0 - 0.1) * np.cos(math.pi * (1.0 - ts) * 0.5) ** 2 + 0.1
    c_t = 1.0 - np.roll(beta, 1) / T
    return dict(c_t=[float(x) for x in c_t])


def _build_module(nop=False, reps=1, consts=None):
    import concourse.tile as tile
    from concourse import bacc, mybir

    if consts is None:
        consts = _default_consts()

    global _SQDIFF_OP
    if KNOB["slots"] == "sqdiff":
        _SQDIFF_OP = _install_sqdiff()

    f32 = mybir.dt.float32
    bf16 = mybir.dt.bfloat16
    f8e4 = mybir.dt.float8e4
    f8e5 = mybir.dt.float8e5
    AF = mybir.ActivationFunctionType
    ALU = mybir.AluOpType

    nc = bacc.Bacc("TRN2", target_bir_lowering=False, debug=False,
                   num_devices=NCORES)

    epsd = nc.dram_tensor("epsd", [T, 128, BS], bf16, kind="ExternalInput").ap()
    z0bd = nc.dram_tensor("z0bd", [128, BS], bf16, kind="ExternalInput").ap()
    z0f8d = nc.dram_tensor("z0f8d", [128, BS], f8e4, kind="ExternalInput").ap()
    mubd = nc.dram_tensor("mubd", [128, BS], bf16, kind="ExternalInput").ap()
    c1f8d = nc.dram_tensor("c1f8d", [4, 128, BS], f8e4, kind="ExternalInput").ap()
    w1drd = nc.dram_tensor("w1drd", [128, 4, 2, 128], f8e5, kind="ExternalInput").ap()
    w2drd = nc.dram_tensor("w2drd", [128, 2, 2, 128], f8e5, kind="ExternalInput").ap()
    identbd = nc.dram_tensor("identbd", [128, 128], bf16, kind="ExternalInput").ap()
    aidd = nc.dram_tensor("aidd", [128, T, 128], bf16, kind="ExternalInput").ap()
    tbld = nc.dram_tensor("tbld", [128, 64], f32, kind="ExternalInput").ap()
    tetd = nc.dram_tensor("tetd", [128, 128], f32, kind="ExternalInput").ap()
    outd = nc.dram_tensor("outd", [128, 2 * T + NP], f32, kind="ExternalOutput").ap()

    with tile.TileContext(nc) as tc:
        with (
            tc.tile_pool(name="const", bufs=1) as cpool,
            tc.tile_pool(name="state", bufs=1) as spool,
            tc.tile_pool(name="eps", bufs=KNOB["eps_bufs"]) as epool,
            tc.tile_pool(name="hs", bufs=KNOB["hs_bufs"]) as hpool,
            tc.tile_pool(name="scr", bufs=2) as scrp,
            tc.tile_pool(name="psH", bufs=KNOB["psh_bufs"], space="PSUM") as psH,
            tc.tile_pool(name="psZ", bufs=KNOB["psz_bufs"], space="PSUM") as psZ,
        ):
            if nop:
                out2 = spool.tile([128, 2 * T + NP], f32, tag="out2")
                nc.gpsimd.memset(out2[:], 0.0)
                nc.sync.dma_start(outd, out2[:])
            elif reps == 1:
                _emit(nc, tc, cpool, spool, epool, hpool, scrp, psH, psZ,
                      f32, bf16, f8e4, f8e5, AF, ALU,
                      epsd, z0bd, z0f8d, mubd, c1f8d, w1drd, w2drd, identbd,
                      aidd, tbld, tetd, outd, consts)
            else:
                with tc.For_i(0, reps, 1):
                    _emit(nc, tc, cpool, spool, epool, hpool, scrp, psH, psZ,
                          f32, bf16, f8e4, f8e5, AF, ALU,
                          epsd, z0bd, z0f8d, mubd, c1f8d, w1drd, w2drd,
                          identbd, aidd, tbld, tetd, outd, consts)

    nc.compile()
    return nc


def _emit(nc, tc, cpool, spool, epool, hpool, scrp, psH, psZ,
          f32, bf16, f8e4, f8e5, AF, ALU,
          epsd, z0bd, z0f8d, mubd, c1f8d, w1drd, w2drd, identbd, aidd, tbld,
          tetd, outd, consts):
    from concourse import mybir
    DR = mybir.MatmulPerfMode.DoubleRow
    c_t = consts["c_t"]          # python floats, len T
    variant = KNOB["variant"]

    # ---- resident constants ----
    w1dr = cpool.tile([128, 4, 2, 128], f8e5, tag="w1dr")
    nc.sync.dma_start(w1dr[:], w1drd)
    w2dr = cpool.tile([128, 2, 2, 128], f8e5, tag="w2dr")
    nc.sync.dma_start(w2dr[:], w2drd)
    identb = cpool.tile([128, 128], bf16, tag="identb")
    nc.sync.dma_start(identb[:], identbd)
    tbl = cpool.tile([128, 64], f32, tag="tbl")
    nc.sync.dma_start(tbl[:], tbld)
    tet = cpool.tile([128, 128], f32, tag="tet")
    nc.sync.dma_start(tet[:], tetd)
    if variant == "pe":
        aid = cpool.tile([128, T, 128], bf16, tag="aid")
        nc.sync.dma_start(aid[:], aidd)

    # ---- state ----
    zc1 = spool.tile([128, 5, BS], f8e4, tag="zc1")
    nc.sync.dma_start(zc1[:, 0, :], z0f8d)
    nc.sync.dma_start(zc1[:, 1:5, :], c1f8d.rearrange("h p b -> p h b"))
    zA = spool.tile([128, BS], bf16, tag="zA")
    zB = spool.tile([128, BS], bf16, tag="zB")
    nc.sync.dma_start(zA[:], z0bd)
    mub = spool.tile([128, BS], bf16, tag="mub")
    nc.sync.dma_start(mub[:], mubd)
    slots = spool.tile([128, 2 * T + NP], f32, tag="slots")
    zbuf = [zA, zB]

    def relu_one(np_, h, hs, hp, t):
        tecol = tet[:, h * 32 + t: h * 32 + t + 1]
        if (np_, h) in KNOB["relu_dve"]:
            nc.vector.tensor_scalar(hs[:, h, :], hp[:], scalar1=tecol,
                                    scalar2=0.0, op0=ALU.add, op1=ALU.max)
        else:
            nc.scalar.activation(hs[:, h, :], hp[:], AF.Relu,
                                 bias=tecol, scale=1.0)

    # ---- main loop (fully unrolled) ----
    for t in range(T):
        zin = zbuf[t % 2]
        zout = zbuf[(t + 1) % 2]
        acol = tbl[:, t:t + 1]
        ept = epool.tile([128, BS], bf16, tag="eps")
        nc.sync.dma_start(ept[:], epsd[t])

        hss = []
        for np_ in range(NP):
            hs = hpool.tile([128, 4, CH], f8e4, tag=f"hs{np_}")
            hss.append(hs)

        # --- W1 + relu ---
        def w1_pair(np_, h):
            base = np_ * CH
            hp = psH.tile([128, CH], f32, tag="hp")
            rhs0 = zc1[:, 0:h + 2:h + 1, base:base + 512]
            rhs1 = zc1[:, 0:h + 2:h + 1, base + 512:base + CH]
            nc.tensor.matmul(hp[:, 0:512], lhsT=w1dr[:, h], rhs=rhs0,
                             start=True, stop=True, perf_mode=DR)
            nc.tensor.matmul(hp[:, 512:CH], lhsT=w1dr[:, h], rhs=rhs1,
                             start=True, stop=True, perf_mode=DR)
            relu_one(np_, h, hss[np_], hp, t)

        if KNOB["hmajor"]:
            for h in range(4):
                for np_ in range(NP):
                    w1_pair(np_, h)
        else:
            for np_ in range(NP):
                for h in range(4):
                    w1_pair(np_, h)

        # --- zps + state + slots per chunk ---
        for np_ in range(NP):
            base = np_ * CH
            nsl2 = slice(base, base + CH)
            hs = hss[np_]
            zps = psZ.tile([128, CH], f32, tag="zps")
            halves = [(slice(hh * 512, (hh + 1) * 512),
                       slice(base + hh * 512, base + (hh + 1) * 512))
                      for hh in range(2)]
            if variant == "pe":
                for osl, zsl in halves:
                    nc.tensor.matmul(zps[:, osl], lhsT=aid[:, t],
                                     rhs=zin[:, zsl], start=True, stop=False)
                for osl, zsl in halves:
                    nc.tensor.matmul(zps[:, osl], lhsT=identb[:],
                                     rhs=ept[:, zsl], start=False, stop=False)
                st0 = False
            else:
                for osl, zsl in halves:
                    nc.tensor.matmul(zps[:, osl], lhsT=identb[:],
                                     rhs=ept[:, zsl], start=True, stop=False)
                st0 = False
            for osl, _ in halves:
                nc.tensor.matmul(zps[:, osl], lhsT=w2dr[:, 0],
                                 rhs=hs[:, 0:2, osl], start=st0, stop=False,
                                 perf_mode=DR)
            for osl, _ in halves:
                nc.tensor.matmul(zps[:, osl], lhsT=w2dr[:, 1],
                                 rhs=hs[:, 2:4, osl], start=False, stop=True,
                                 perf_mode=DR)

            import contextlib
            hi = tc.high_priority() if KNOB["hi_zc1"] else contextlib.nullcontext()
            if variant == "pe":
                # zc1 fp8 = cast(zps) straight from PSUM (critical path)
                ev = ([(slice(0, 512), slice(base, base + 512)),
                       (slice(512, CH), slice(base + 512, base + CH))]
                      if KNOB["zc1_halves"] else [(slice(0, CH), nsl2)])
                with hi:
                    for osl, zsl in ev:
                        if KNOB["zc1_eng"] == "act":
                            nc.scalar.copy(zc1[:, 0, zsl], zps[:, osl])
                        else:
                            nc.vector.tensor_copy(zc1[:, 0, zsl], zps[:, osl])
                # bf16 state evac (off critical path)
                if KNOB["zout_eng"] == "dve":
                    nc.vector.tensor_copy(zout[:, nsl2], zps[:])
                else:
                    nc.scalar.copy(zout[:, nsl2], zps[:])
            else:
                # z' = a_t z + zps on DVE (halves), then fp8 cast
                with hi:
                    for osl, zsl in halves:
                        nc.vector.scalar_tensor_tensor(
                            zout[:, zsl], in0=zin[:, zsl], scalar=acol,
                            in1=zps[:, osl], op0=ALU.mult, op1=ALU.add)
                    for osl, zsl in halves:
                        if KNOB["cast_eng"] == "pool":
                            nc.gpsimd.tensor_copy(zc1[:, 0, zsl], zout[:, zsl])
                        elif KNOB["cast_eng"] == "act":
                            nc.scalar.copy(zc1[:, 0, zsl], zout[:, zsl])
                        else:
                            nc.vector.tensor_copy(zc1[:, 0, zsl], zout[:, zsl])

            # --- slots: sum_b (zin - c*zout)^2 ---
            col = slots[:, 2 * t + np_: 2 * t + np_ + 1]
            if KNOB["slots"] == "sqdiff":
                so = scrp.tile([128, CH], bf16, tag=f"scrSo{np_}")
                nc.vector._custom_dve(
                    _SQDIFF_OP, out=so[:], in0=zout[:, nsl2],
                    in1=zin[:, nsl2], s0=float(-c_t[t]), accum_out=col)
                continue
            w = scrp.tile([128, CH], bf16, tag=f"scrW{np_}")
            if KNOB["w_eng"] == "pool":
                nc.gpsimd.tensor_scalar(w[:], zout[:, nsl2],
                                        scalar1=float(-c_t[t]), scalar2=None,
                                        op0=ALU.mult)
            else:
                nc.vector.tensor_scalar(w[:], zout[:, nsl2],
                                        scalar1=float(-c_t[t]), scalar2=None,
                                        op0=ALU.mult)
            v = scrp.tile([128, CH], bf16, tag=f"scrV{np_}")
            if KNOB["v_eng"] == "pool":
                nc.gpsimd.tensor_tensor(v[:], w[:], zin[:, nsl2], op=ALU.add)
            else:
                nc.vector.tensor_add(v[:], w[:], zin[:, nsl2])
            if KNOB["sq_eng"] == "act":
                sd = scrp.tile([128, 1], f32, tag=f"scrA{np_}")
                nc.scalar.activation(sd[:].broadcast_to((128, CH)), v[:],
                                     AF.Square, bias=0.0, scale=1.0,
                                     accum_out=col)
            else:
                so = scrp.tile([128, CH], bf16, tag=f"scrSo{np_}")
                nc.vector.tensor_tensor_reduce(
                    out=so[:], in0=v[:], in1=v[:], scale=1.0, scalar=0.0,
                    op0=ALU.mult, op1=ALU.add, accum_out=col)

    # ---- terminal: slots[64+np] = sum_b (z_T - mu)^2 ----
    zfin = zbuf[T % 2]
    for np_ in range(NP):
        nsl2 = slice(np_ * CH, (np_ + 1) * CH)
        vT = scrp.tile([128, CH], bf16, tag=f"scrV{np_}")
        nc.vector.tensor_sub(vT[:], zfin[:, nsl2], mub[:, nsl2])
        sT = scrp.tile([128, 1], f32, tag=f"scrA{np_}")
        nc.vector.scalar_tensor_tensor(
            sT[:].broadcast_to((128, CH)), in0=vT[:], scalar=1.0, in1=vT[:],
            op0=ALU.mult, op1=ALU.mult,
            accum_out=slots[:, 2 * T + np_: 2 * T + np_ + 1])

    nc.sync.dma_start(outd, slots[:])


def _host_prep(inputs):
    """Numpy-only preprocessing: dtype casts, transposes, shards, tables.

    Returns (in_maps, host_terms, consts).
    """
    import ml_dtypes
    bf16 = ml_dtypes.bfloat16
    f8e4 = ml_dtypes.float8_e4m3fn
    f8e5 = ml_dtypes.float8_e5m2

    ctx = np.asarray(inputs["context_embedding"], np.float32)
    eps0 = np.asarray(inputs["eps0"], np.float32)
    eps = np.asarray(inputs["eps"], np.float32)
    beta = np.asarray(inputs["beta_schedule"], np.float64)
    sig0 = float(np.asarray(inputs["sigma0"], np.float32)[0])
    W1 = np.asarray(inputs["W1"], np.float32)
    b1 = np.asarray(inputs["b1"], np.float32)
    W2 = np.asarray(inputs["W2"], np.float32)
    b2 = np.asarray(inputs["b2"], np.float32)
    te = np.asarray(inputs["t_emb"], np.float32)
    mu = np.asarray(inputs["target_mu"], np.float32)

    dt = 1.0 / T
    bb = np.roll(beta, 1)
    a_t = 1.0 + beta * dt
    c_t = 1.0 - bb * dt
    s_t = np.sqrt(2.0 * beta * dt) * sig0
    sb_t = np.sqrt(2.0 * bb * dt) * sig0
    k_t = 0.5 / sb_t ** 2
    const_per = float(np.sum(np.log(s_t) - np.log(sb_t)))

    if np.any(b2):
        raise NotImplementedError("nonzero b2 not supported by this kernel")

    # pre-scaled noise eps' = bf16(s_t * eps), transposed to [T, Z, B]
    epsb = (eps * s_t[:, None, None].astype(np.float32)).astype(bf16)
    host_eps = 0.0
    for t in range(T):
        host_eps += 0.5 * float(
            (epsb[t].astype(np.float32) ** 2).sum(dtype=np.float64)
        ) / float(s_t[t] ** 2)
    epsb_T = np.ascontiguousarray(epsb.transpose(0, 2, 1))  # [T, Z, B]

    z0b = (np.float32(sig0) * eps0).astype(bf16)            # [B, Z]
    host_e0 = 0.5 * float(
        (z0b.astype(np.float32) ** 2).sum(dtype=np.float64)
    ) / (sig0 ** 2)
    z0b_T = np.ascontiguousarray(z0b.T)                     # [Z, B]
    z0f8_T = z0b_T.astype(f8e4)
    mub_T = np.ascontiguousarray(mu.T.astype(bf16))         # [Z, B]

    c1 = (ctx @ W1[Z:] + b1).astype(np.float32)             # [B, HID]
    c1f8 = np.ascontiguousarray(c1.T).reshape(4, 128, B).astype(f8e4)

    # W1-DoubleRow fused weights: [ki, h, ko, m]; ko=0 -> W1z, ko=1 -> I
    w1dr = np.zeros((128, 4, 2, 128), np.float32)
    w1z = W1[:Z]                                            # [128, 512]
    idx = np.arange(128)
    for h in range(4):
        w1dr[:, h, 0, :] = w1z[:, h * 128:(h + 1) * 128]
        w1dr[idx, h, 1, idx] = 1.0
    w1dr = w1dr.astype(f8e5)

    # W2-DoubleRow weights: [ki, pair, ko, m] = dt * W2[(2p+ko)*128+ki, m]
    w2s = (W2 * np.float32(dt)).reshape(2, 2, 128, 128)     # [p, ko, ki, m]
    w2dr = np.ascontiguousarray(w2s.transpose(2, 0, 1, 3)).astype(f8e5)

    identb = np.eye(128, dtype=bf16)

    # per-step scaled identities a_t * I for the z-state matmul, [ki, t, m]
    aidd = np.zeros((128, T, 128), np.float32)
    aidd[idx, :, idx] = a_t[None, :].astype(np.float32)
    aidd = aidd.astype(bf16)

    tbl = np.zeros((128, 64), np.float32)
    tbl[:, 0:T] = a_t[None, :].astype(np.float32)
    tbl[:, 32:64] = -c_t[None, :].astype(np.float32)

    tet = np.zeros((128, 128), np.float32)
    for h in range(4):
        tet[:, h * 32:(h + 1) * 32] = te[:, h * 128:(h + 1) * 128].T

    in_maps = []
    for c in range(NCORES):
        bs = slice(c * BS, (c + 1) * BS)
        in_maps.append({
            "epsd": np.ascontiguousarray(epsb_T[:, :, bs]),
            "z0bd": np.ascontiguousarray(z0b_T[:, bs]),
            "z0f8d": np.ascontiguousarray(z0f8_T[:, bs]),
            "mubd": np.ascontiguousarray(mub_T[:, bs]),
            "c1f8d": np.ascontiguousarray(c1f8[:, :, bs]),
            "w1drd": w1dr,
            "w2drd": w2dr,
            "identbd": identb,
            "aidd": aidd,
            "tbld": tbl,
            "tetd": tet,
        })
    host_terms = dict(
        host_sum=host_eps + host_e0
        + B * Z * (const_per + math.log(sig0)),
        k_t=k_t,
    )
    consts = dict(c_t=[float(x) for x in c_t])
    return in_maps, host_terms, consts


def _assemble(results, host_terms):
    """Combine per-core slot outputs with the host terms."""
    k_t = host_terms["k_t"]
    dev = 0.0
    for res in results:
        o = res["outd"].astype(np.float64)                  # [128, 66]
        st = o[:, 0:2 * T].reshape(128, T, NP).sum(axis=2)  # [128, T]
        dev += float((st * k_t[None, :]).sum()) + 0.5 * float(
            o[:, 2 * T:].sum())
    total = (host_terms["host_sum"] - dev) / B
    return np.float32(total)


def _install_neff_cache():
    """Cache walrus NEFF output by BIR hash (compile takes minutes otherwise)."""
    import hashlib
    import os
    import shutil

    from concourse import bass2jax

    if getattr(bass2jax, "_ant_neff_cache_installed", False):
        return
    orig = bass2jax.compile_bir_kernel
    cache_dir = os.environ.get("BASS_NEFF_CACHE", "/tmp/neff_cache")

    def cached(bir_json, tmpdir, neff_name="file.neff"):
        os.makedirs(cache_dir, exist_ok=True)
        key = hashlib.sha256(bir_json if isinstance(bir_json, bytes)
                             else bir_json.encode()).hexdigest()[:24]
        hit = os.path.join(cache_dir, f"{key}.neff")
        dst = os.path.join(tmpdir, neff_name)
        if os.path.exists(hit):
            shutil.copy(hit, dst)
            return dst
        out = orig(bir_json, tmpdir, neff_name)
        shutil.copy(out, hit)
        return out

    bass2jax.compile_bir_kernel = cached
    bass2jax._ant_neff_cache_installed = True


def kernel(**inputs) -> np.ndarray:
    from concourse import bass_utils

    _install_neff_cache()
    in_maps, host_terms, consts = _host_prep(inputs)
    key = ("nc", tuple(consts["c_t"]))
    if key not in _cache:
        _cache[key] = _build_module(consts=consts)
        _cache["nc"] = _cache[key]
    nc = _cache[key]

    res = bass_utils.run_bass_kernel_spmd(nc, in_maps, core_ids=list(range(NCORES)))
    _cache["last_res"] = res
    return _assemble(res.results, host_terms)


# revision 18
# speedup vs baseline: 1.3946x; 1.3946x over previous
"""Trainium2 Bass kernel for the diffusion-sampler importance-weight problem.

Math (per batch element b, per z-dim p), derived from the reference:
  z_0 = sigma0 * eps0
  per step t (beta_f = beta[t], beta_b = roll(beta,1)[t]):
    hid   = relu(W1z^T z + c1 + te_t)        c1 = ctx @ W1[Z:] + b1 (host)
    u'    = dt * W2^T hid
    z'    = a_t z + u' + eps'_t              a_t = 1 + beta_f dt,
                                             eps' = bf16(s_t eps), s=sqrt(2 b dt) s0
    logw += 0.5 (eps'/s_t)^2 - k_t (z - c_t z')^2 + log(s_t/sb_t)
            c_t = 1 - beta_b dt, sb = sqrt(2 beta_b dt) s0, k = 0.5/sb^2
  terminal: logw += -0.5 (z_T - mu)^2 + 0.5 eps0^2 + log(sig0)
  output = sum_z mean_b logw

Device computes ONLY the trajectory-dependent sums per (p, t, halfchunk):
  slots[p, 2t+j]  = sum_{b in chunk j} (z_prev - c_t z_next)^2
  slots[p, 64+j]  = sum_{b in chunk j} (z_T - mu)^2
Host adds the noise/constant terms and the k_t weighting.

Device layout: feature-major [Z=128 partitions, batch free], batch sharded
8 ways (BS=2048/core, NP=2 chunks of 1024). Matmuls fp8 DoubleRow (K=256):
  hp  = [W1z; I]^T [z; c1]     (c1-add fused via identity rows)
  zps = a_t z (scaled-ident bf16 MM, variant "pe") + dt*W2^T hs + I^T eps'
W1 matmuls are emitted h-major across the two batch chunks so each DR weight
is loaded once per step.  State z is bf16; zc1 fp8 feeds the next W1.
"""
import math
import numpy as np

B, Z, H, HID, T = 16384, 128, 512, 512, 32
NCORES = 8
BS = B // NCORES          # 2048 batch rows per core
NP = 2                    # batch chunks of 1024 per core
CH = BS // NP             # 1024

_cache: dict = {}
# tuning knobs
KNOB = dict(
    variant="stt",               # pe: z' accumulated in PSUM via a_t-ident MM
                                 # stt: z' via DVE STT (baseline-style)
    relu_dve={(0, 3), (1, 3)},   # (np, h) relu chunks on DVE; rest ACT
    zc1_eng="act",               # fp8 z for next W1: act | dve   (variant pe)
    zc1_halves=True,             # emit zc1 evac as 2x512 halves
    zout_eng="dve",              # bf16 state evac (variant pe): dve | act
    cast_eng="pool",             # variant stt: fp8 cast engine pool|act|dve
    slots="sqdiff",              # sqdiff: fused custom-DVE sum((zin-c*zout)^2)
                                 # wvsq: w=TS, v=TT, sq per sq_eng
    w_eng="dve",                 # w = -c*zout:  pool | dve
    v_eng="dve",                 # v = w + zin:  dve | pool
    sq_eng="act",                # act (Square accum) | dve (TTR custom)
    hmajor=False,                # W1 MMs h-major across chunks (shared LDW)
    eps_bufs=3, hs_bufs=2, psh_bufs=2, psz_bufs=2,
    hi_zc1=True,                 # high_priority on zc1/cast evac
)


def _install_sqdiff():
    """Register a fused custom-DVE op: out = (in0*c0 + in1)^2, accum = sum.

    Computes sum_b (zin - c*zout)^2 in one DVE pass (replaces TS+TT+Square).
    The uop tables are written per-NEFF, so registration here is all that's
    needed; the sha pin is derived by a trial compile.
    """
    import operator
    import re

    from concourse import dve_ops
    from concourse.dve_spec import C0, Spec, Src0, Src1, Zero, sq

    for op in dve_ops.OPS:
        if op.name == "SQDIFF_ACC_ANT":
            return op
    def _ref(in0, in1, c0, c1, c2):
        b = (in0.astype(np.float32) * c0 + in1.astype(np.float32)) ** 2
        return b, b.reshape(b.shape[0], -1).sum(axis=-1, keepdims=True)

    spec = Spec(body=sq(Src0 * C0 + Src1), accum=operator.add,
                accum_init=Zero, reference=_ref)
    op = dve_ops.DveOp("SQDIFF_ACC_ANT", spec, subdim=False, uops_sha={})
    dve_ops._SUB_OPCODE_FOR_NAME[op.name] = (
        max(dve_ops._SUB_OPCODE_FOR_NAME.values()) + 1)
    for ver in ("v3", "v4"):
        try:
            dve_ops._COMPILE_CACHE.pop((op.name, ver), None)
            op.compile(ver)
        except ValueError as e:
            m = re.search(r'\["{}"\]="([0-9a-f]+)"'.format(ver), str(e))
            assert m, f"could not pin sha for {ver}: {e}"
            op.uops_sha[ver] = m.group(1)
            dve_ops._COMPILE_CACHE.pop((op.name, ver), None)
    dve_ops.OPS.append(op)
    dve_ops.CUSTOM_DVE_SPECS[op.name] = spec
    return op


def _default_consts():
    """c_t from the deterministic cosine beta schedule (matches setup_inputs)."""
    ts = np.linspace(1.0, 0.0, T)
    beta = (1.0 - 0.1) * np.cos(math.pi * (1.0 - ts) * 0.5) ** 2 + 0.1
    c_t = 1.0 - np.roll(beta, 1) / T
    return dict(c_t=[float(x) for x in c_t])


def _build_module(nop=False, reps=1, consts=None):
    import concourse.tile as tile
    from concourse import bacc, mybir

    if consts is None:
        consts = _default_consts()

    global _SQDIFF_OP
    if KNOB["slots"] == "sqdiff":
        _SQDIFF_OP = _install_sqdiff()

    f32 = mybir.dt.float32
    bf16 = mybir.dt.bfloat16
    f8e4 = mybir.dt.float8e4
    f8e5 = mybir.dt.float8e5
    AF = mybir.ActivationFunctionType
    ALU = mybir.AluOpType

    nc = bacc.Bacc("TRN2", target_bir_lowering=False, debug=False,
                   num_devices=NCORES)

    epsd = nc.dram_tensor("epsd", [T, 128, BS], bf16, kind="ExternalInput").ap()
    z0bd = nc.dram_tensor("z0bd", [128, BS], bf16, kind="ExternalInput").ap()
    z0f8d = nc.dram_tensor("z0f8d", [128, BS], f8e4, kind="ExternalInput").ap()
    mubd = nc.dram_tensor("mubd", [128, BS], bf16, kind="ExternalInput").ap()
    c1f8d = nc.dram_tensor("c1f8d", [4, 128, BS], f8e4, kind="ExternalInput").ap()
    w1drd = nc.dram_tensor("w1drd", [128, 4, 2, 128], f8e5, kind="ExternalInput").ap()
    w2drd = nc.dram_tensor("w2drd", [128, 2, 2, 128], f8e5, kind="ExternalInput").ap()
    identbd = nc.dram_tensor("identbd", [128, 128], bf16, kind="ExternalInput").ap()
    aidd = nc.dram_tensor("aidd", [128, T, 128], bf16, kind="ExternalInput").ap()
    tbld = nc.dram_tensor("tbld", [128, 64], f32, kind="ExternalInput").ap()
    tetd = nc.dram_tensor("tetd", [128, 128], f32, kind="ExternalInput").ap()
    outd = nc.dram_tensor("outd", [128, 2 * T + NP], f32, kind="ExternalOutput").ap()

    with tile.TileContext(nc) as tc:
        with (
            tc.tile_pool(name="const", bufs=1) as cpool,
            tc.tile_pool(name="state", bufs=1) as spool,
            tc.tile_pool(name="eps", bufs=KNOB["eps_bufs"]) as epool,
            tc.tile_pool(name="hs", bufs=KNOB["hs_bufs"]) as hpool,
            tc.tile_pool(name="scr", bufs=2) as scrp,
            tc.tile_pool(name="psH", bufs=KNOB["psh_bufs"], space="PSUM") as psH,
            tc.tile_pool(name="psZ", bufs=KNOB["psz_bufs"], space="PSUM") as psZ,
        ):
            if nop:
                out2 = spool.tile([128, 2 * T + NP], f32, tag="out2")
                nc.gpsimd.memset(out2[:], 0.0)
                nc.sync.dma_start(outd, out2[:])
            elif reps == 1:
                _emit(nc, tc, cpool, spool, epool, hpool, scrp, psH, psZ,
                      f32, bf16, f8e4, f8e5, AF, ALU,
                      epsd, z0bd, z0f8d, mubd, c1f8d, w1drd, w2drd, identbd,
                      aidd, tbld, tetd, outd, consts)
            else:
                with tc.For_i(0, reps, 1):
                    _emit(nc, tc, cpool, spool, epool, hpool, scrp, psH, psZ,
                          f32, bf16, f8e4, f8e5, AF, ALU,
                          epsd, z0bd, z0f8d, mubd, c1f8d, w1drd, w2drd,
                          identbd, aidd, tbld, tetd, outd, consts)

    nc.compile()
    return nc


def _emit(nc, tc, cpool, spool, epool, hpool, scrp, psH, psZ,
          f32, bf16, f8e4, f8e5, AF, ALU,
          epsd, z0bd, z0f8d, mubd, c1f8d, w1drd, w2drd, identbd, aidd, tbld,
          tetd, outd, consts):
    from concourse import mybir
    DR = mybir.MatmulPerfMode.DoubleRow
    c_t = consts["c_t"]          # python floats, len T
    variant = KNOB["variant"]

    # ---- resident constants ----
    w1dr = cpool.tile([128, 4, 2, 128], f8e5, tag="w1dr")
    nc.sync.dma_start(w1dr[:], w1drd)
    w2dr = cpool.tile([128, 2, 2, 128], f8e5, tag="w2dr")
    nc.sync.dma_start(w2dr[:], w2drd)
    identb = cpool.tile([128, 128], bf16, tag="identb")
    nc.sync.dma_start(identb[:], identbd)
    tbl = cpool.tile([128, 64], f32, tag="tbl")
    nc.sync.dma_start(tbl[:], tbld)
    tet = cpool.tile([128, 128], f32, tag="tet")
    nc.sync.dma_start(tet[:], tetd)
    if variant == "pe":
        aid = cpool.tile([128, T, 128], bf16, tag="aid")
        nc.sync.dma_start(aid[:], aidd)

    # ---- state ----
    zc1 = spool.tile([128, 5, BS], f8e4, tag="zc1")
    nc.sync.dma_start(zc1[:, 0, :], z0f8d)
    nc.sync.dma_start(zc1[:, 1:5, :], c1f8d.rearrange("h p b -> p h b"))
    zA = spool.tile([128, BS], bf16, tag="zA")
    zB = spool.tile([128, BS], bf16, tag="zB")
    nc.sync.dma_start(zA[:], z0bd)
    mub = spool.tile([128, BS], bf16, tag="mub")
    nc.sync.dma_start(mub[:], mubd)
    slots = spool.tile([128, 2 * T + NP], f32, tag="slots")
    zbuf = [zA, zB]

    def relu_one(np_, h, hs, hp, t):
        tecol = tet[:, h * 32 + t: h * 32 + t + 1]
        if (np_, h) in KNOB["relu_dve"]:
            nc.vector.tensor_scalar(hs[:, h, :], hp[:], scalar1=tecol,
                                    scalar2=0.0, op0=ALU.add, op1=ALU.max)
        else:
            nc.scalar.activation(hs[:, h, :], hp[:], AF.Relu,
                                 bias=tecol, scale=1.0)

    # ---- main loop (fully unrolled) ----
    for t in range(T):
        zin = zbuf[t % 2]
        zout = zbuf[(t + 1) % 2]
        acol = tbl[:, t:t + 1]
        ept = epool.tile([128, BS], bf16, tag="eps")
        nc.sync.dma_start(ept[:], epsd[t])

        hss = []
        for np_ in range(NP):
            hs = hpool.tile([128, 4, CH], f8e4, tag=f"hs{np_}")
            hss.append(hs)

        # --- W1 + relu ---
        def w1_pair(np_, h):
            base = np_ * CH
            hp = psH.tile([128, CH], f32, tag="hp")
            rhs0 = zc1[:, 0:h + 2:h + 1, base:base + 512]
            rhs1 = zc1[:, 0:h + 2:h + 1, base + 512:base + CH]
            nc.tensor.matmul(hp[:, 0:512], lhsT=w1dr[:, h], rhs=rhs0,
                             start=True, stop=True, perf_mode=DR)
            nc.tensor.matmul(hp[:, 512:CH], lhsT=w1dr[:, h], rhs=rhs1,
                             start=True, stop=True, perf_mode=DR)
            relu_one(np_, h, hss[np_], hp, t)

        if KNOB["hmajor"]:
            for h in range(4):
                for np_ in range(NP):
                    w1_pair(np_, h)
        else:
            for np_ in range(NP):
                for h in range(4):
                    w1_pair(np_, h)

        # --- zps + state + slots per chunk ---
        for np_ in range(NP):
            base = np_ * CH
            nsl2 = slice(base, base + CH)
            hs = hss[np_]
            zps = psZ.tile([128, CH], f32, tag="zps")
            halves = [(slice(hh * 512, (hh + 1) * 512),
                       slice(base + hh * 512, base + (hh + 1) * 512))
                      for hh in range(2)]
            if variant == "pe":
                for osl, zsl in halves:
                    nc.tensor.matmul(zps[:, osl], lhsT=aid[:, t],
                                     rhs=zin[:, zsl], start=True, stop=False)
                for osl, zsl in halves:
                    nc.tensor.matmul(zps[:, osl], lhsT=identb[:],
                                     rhs=ept[:, zsl], start=False, stop=False)
                st0 = False
            else:
                for osl, zsl in halves:
                    nc.tensor.matmul(zps[:, osl], lhsT=identb[:],
                                     rhs=ept[:, zsl], start=True, stop=False)
                st0 = False
            for osl, _ in halves:
                nc.tensor.matmul(zps[:, osl], lhsT=w2dr[:, 0],
                                 rhs=hs[:, 0:2, osl], start=st0, stop=False,
                                 perf_mode=DR)
            for osl, _ in halves:
                nc.tensor.matmul(zps[:, osl], lhsT=w2dr[:, 1],
                                 rhs=hs[:, 2:4, osl], start=False, stop=True,
                                 perf_mode=DR)

            import contextlib
            hi = tc.high_priority() if KNOB["hi_zc1"] else contextlib.nullcontext()
            if variant == "pe":
                # zc1 fp8 = cast(zps) straight from PSUM (critical path)
                ev = ([(slice(0, 512), slice(base, base + 512)),
                       (slice(512, CH), slice(base + 512, base + CH))]
                      if KNOB["zc1_halves"] else [(slice(0, CH), nsl2)])
                with hi:
                    for osl, zsl in ev:
                        if KNOB["zc1_eng"] == "act":
                            nc.scalar.copy(zc1[:, 0, zsl], zps[:, osl])
                        else:
                            nc.vector.tensor_copy(zc1[:, 0, zsl], zps[:, osl])
                # bf16 state evac (off critical path)
                if KNOB["zout_eng"] == "dve":
                    nc.vector.tensor_copy(zout[:, nsl2], zps[:])
                else:
                    nc.scalar.copy(zout[:, nsl2], zps[:])
            else:
                # z' = a_t z + zps on DVE (halves), then fp8 cast
                with hi:
                    for osl, zsl in halves:
                        nc.vector.scalar_tensor_tensor(
                            zout[:, zsl], in0=zin[:, zsl], scalar=acol,
                            in1=zps[:, osl], op0=ALU.mult, op1=ALU.add)
                    for osl, zsl in halves:
                        if KNOB["cast_eng"] == "pool":
                            nc.gpsimd.tensor_copy(zc1[:, 0, zsl], zout[:, zsl])
                        elif KNOB["cast_eng"] == "act":
                            nc.scalar.copy(zc1[:, 0, zsl], zout[:, zsl])
                        else:
                            nc.vector.tensor_copy(zc1[:, 0, zsl], zout[:, zsl])

            # --- slots: sum_b (zin - c*zout)^2 ---
            col = slots[:, 2 * t + np_: 2 * t + np_ + 1]
            if KNOB["slots"] == "sqdiff":
                so = scrp.tile([128, CH], bf16, tag=f"scrSo{np_}")
                nc.vector._custom_dve(
                    _SQDIFF_OP, out=so[:], in0=zout[:, nsl2],
                    in1=zin[:, nsl2], s0=float(-c_t[t]), accum_out=col)
                continue
            w = scrp.tile([128, CH], bf16, tag=f"scrW{np_}")
            if KNOB["w_eng"] == "pool":
                nc.gpsimd.tensor_scalar(w[:], zout[:, nsl2],
                                        scalar1=float(-c_t[t]), scalar2=None,
                                        op0=ALU.mult)
            else:
                nc.vector.tensor_scalar(w[:], zout[:, nsl2],
                                        scalar1=float(-c_t[t]), scalar2=None,
                                        op0=ALU.mult)
            v = scrp.tile([128, CH], bf16, tag=f"scrV{np_}")
            if KNOB["v_eng"] == "pool":
                nc.gpsimd.tensor_tensor(v[:], w[:], zin[:, nsl2], op=ALU.add)
            else:
                nc.vector.tensor_add(v[:], w[:], zin[:, nsl2])
            if KNOB["sq_eng"] == "act":
                sd = scrp.tile([128, 1], f32, tag=f"scrA{np_}")
                nc.scalar.activation(sd[:].broadcast_to((128, CH)), v[:],
                                     AF.Square, bias=0.0, scale=1.0,
                                     accum_out=col)
            else:
                so = scrp.tile([128, CH], bf16, tag=f"scrSo{np_}")
                nc.vector.tensor_tensor_reduce(
                    out=so[:], in0=v[:], in1=v[:], scale=1.0, scalar=0.0,
                    op0=ALU.mult, op1=ALU.add, accum_out=col)

    # ---- terminal: slots[64+np] = sum_b (z_T - mu)^2 ----
    zfin = zbuf[T % 2]
    for np_ in range(NP):
        nsl2 = slice(np_ * CH, (np_ + 1) * CH)
        vT = scrp.tile([128, CH], bf16, tag=f"scrV{np_}")
        nc.vector.tensor_sub(vT[:], zfin[:, nsl2], mub[:, nsl2])
        sT = scrp.tile([128, 1], f32, tag=f"scrA{np_}")
        nc.vector.scalar_tensor_tensor(
            sT[:].broadcast_to((128, CH)), in0=vT[:], scalar=1.0, in1=vT[:],
            op0=ALU.mult, op1=ALU.mult,
            accum_out=slots[:, 2 * T + np_: 2 * T + np_ + 1])

    nc.sync.dma_start(outd, slots[:])


def _host_prep(inputs):
    """Numpy-only preprocessing: dtype casts, transposes, shards, tables.

    Returns (in_maps, host_terms, consts).
    """
    import ml_dtypes
    bf16 = ml_dtypes.bfloat16
    f8e4 = ml_dtypes.float8_e4m3fn
    f8e5 = ml_dtypes.float8_e5m2

    ctx = np.asarray(inputs["context_embedding"], np.float32)
    eps0 = np.asarray(inputs["eps0"], np.float32)
    eps = np.asarray(inputs["eps"], np.float32)
    beta = np.asarray(inputs["beta_schedule"], np.float64)
    sig0 = float(np.asarray(inputs["sigma0"], np.float32)[0])
    W1 = np.asarray(inputs["W1"], np.float32)
    b1 = np.asarray(inputs["b1"], np.float32)
    W2 = np.asarray(inputs["W2"], np.float32)
    b2 = np.asarray(inputs["b2"], np.float32)
    te = np.asarray(inputs["t_emb"], np.float32)
    mu = np.asarray(inputs["target_mu"], np.float32)

    dt = 1.0 / T
    bb = np.roll(beta, 1)
    a_t = 1.0 + beta * dt
    c_t = 1.0 - bb * dt
    s_t = np.sqrt(2.0 * beta * dt) * sig0
    sb_t = np.sqrt(2.0 * bb * dt) * sig0
    k_t = 0.5 / sb_t ** 2
    const_per = float(np.sum(np.log(s_t) - np.log(sb_t)))

    if np.any(b2):
        raise NotImplementedError("nonzero b2 not supported by this kernel")

    # pre-scaled noise eps' = bf16(s_t * eps), transposed to [T, Z, B]
    epsb = (eps * s_t[:, None, None].astype(np.float32)).astype(bf16)
    host_eps = 0.0
    for t in range(T):
        host_eps += 0.5 * float(
            (epsb[t].astype(np.float32) ** 2).sum(dtype=np.float64)
        ) / float(s_t[t] ** 2)
    epsb_T = np.ascontiguousarray(epsb.transpose(0, 2, 1))  # [T, Z, B]

    z0b = (np.float32(sig0) * eps0).astype(bf16)            # [B, Z]
    host_e0 = 0.5 * float(
        (z0b.astype(np.float32) ** 2).sum(dtype=np.float64)
    ) / (sig0 ** 2)
    z0b_T = np.ascontiguousarray(z0b.T)                     # [Z, B]
    z0f8_T = z0b_T.astype(f8e4)
    mub_T = np.ascontiguousarray(mu.T.astype(bf16))         # [Z, B]

    c1 = (ctx @ W1[Z:] + b1).astype(np.float32)             # [B, HID]
    c1f8 = np.ascontiguousarray(c1.T).reshape(4, 128, B).astype(f8e4)

    # W1-DoubleRow fused weights: [ki, h, ko, m]; ko=0 -> W1z, ko=1 -> I
    w1dr = np.zeros((128, 4, 2, 128), np.float32)
    w1z = W1[:Z]                                            # [128, 512]
    idx = np.arange(128)
    for h in range(4):
        w1dr[:, h, 0, :] = w1z[:, h * 128:(h + 1) * 128]
        w1dr[idx, h, 1, idx] = 1.0
    w1dr = w1dr.astype(f8e5)

    # W2-DoubleRow weights: [ki, pair, ko, m] = dt * W2[(2p+ko)*128+ki, m]
    w2s = (W2 * np.float32(dt)).reshape(2, 2, 128, 128)     # [p, ko, ki, m]
    w2dr = np.ascontiguousarray(w2s.transpose(2, 0, 1, 3)).astype(f8e5)

    identb = np.eye(128, dtype=bf16)

    # per-step scaled identities a_t * I for the z-state matmul, [ki, t, m]
    aidd = np.zeros((128, T, 128), np.float32)
    aidd[idx, :, idx] = a_t[None, :].astype(np.float32)
    aidd = aidd.astype(bf16)

    tbl = np.zeros((128, 64), np.float32)
    tbl[:, 0:T] = a_t[None, :].astype(np.float32)
    tbl[:, 32:64] = -c_t[None, :].astype(np.float32)

    tet = np.zeros((128, 128), np.float32)
    for h in range(4):
        tet[:, h * 32:(h + 1) * 32] = te[:, h * 128:(h + 1) * 128].T

    in_maps = []
    for c in range(NCORES):
        bs = slice(c * BS, (c + 1) * BS)
        in_maps.append({
            "epsd": np.ascontiguousarray(epsb_T[:, :, bs]),
            "z0bd": np.ascontiguousarray(z0b_T[:, bs]),
            "z0f8d": np.ascontiguousarray(z0f8_T[:, bs]),
            "mubd": np.ascontiguousarray(mub_T[:, bs]),
            "c1f8d": np.ascontiguousarray(c1f8[:, :, bs]),
            "w1drd": w1dr,
            "w2drd": w2dr,
            "identbd": identb,
            "aidd": aidd,
            "tbld": tbl,
            "tetd": tet,
        })
    host_terms = dict(
        host_sum=host_eps + host_e0
        + B * Z * (const_per + math.log(sig0)),
        k_t=k_t,
    )
    consts = dict(c_t=[float(x) for x in c_t])
    return in_maps, host_terms, consts


def _assemble(results, host_terms):
    """Combine per-core slot outputs with the host terms."""
    k_t = host_terms["k_t"]
    dev = 0.0
    for res in results:
        o = res["outd"].astype(np.float64)                  # [128, 66]
        st = o[:, 0:2 * T].reshape(128, T, NP).sum(axis=2)  # [128, T]
        dev += float((st * k_t[None, :]).sum()) + 0.5 * float(
            o[:, 2 * T:].sum())
    total = (host_terms["host_sum"] - dev) / B
    return np.float32(total)


def _install_neff_cache():
    """Cache walrus NEFF output by BIR hash (compile takes minutes otherwise)."""
    import hashlib
    import os
    import shutil

    from concourse import bass2jax

    if getattr(bass2jax, "_ant_neff_cache_installed", False):
        return
    orig = bass2jax.compile_bir_kernel
    cache_dir = os.environ.get("BASS_NEFF_CACHE", "/tmp/neff_cache")

    def cached(bir_json, tmpdir, neff_name="file.neff"):
        os.makedirs(cache_dir, exist_ok=True)
        key = hashlib.sha256(bir_json if isinstance(bir_json, bytes)
                             else bir_json.encode()).hexdigest()[:24]
        hit = os.path.join(cache_dir, f"{key}.neff")
        dst = os.path.join(tmpdir, neff_name)
        if os.path.exists(hit):
            shutil.copy(hit, dst)
            return dst
        out = orig(bir_json, tmpdir, neff_name)
        shutil.copy(out, hit)
        return out

    bass2jax.compile_bir_kernel = cached
    bass2jax._ant_neff_cache_installed = True


def kernel(**inputs) -> np.ndarray:
    from concourse import bass_utils

    _install_neff_cache()
    in_maps, host_terms, consts = _host_prep(inputs)
    key = ("nc", tuple(consts["c_t"]))
    if key not in _cache:
        _cache[key] = _build_module(consts=consts)
        _cache["nc"] = _cache[key]
    nc = _cache[key]

    res = bass_utils.run_bass_kernel_spmd(nc, in_maps, core_ids=list(range(NCORES)))
    _cache["last_res"] = res
    return _assemble(res.results, host_terms)


# revision 20
# speedup vs baseline: 1.5930x; 1.1423x over previous
"""Trainium2 Bass kernel for the diffusion-sampler importance-weight problem.

Math (per batch element b, per z-dim p), derived from the reference:
  z_0 = sigma0 * eps0
  per step t (beta_f = beta[t], beta_b = roll(beta,1)[t]):
    hid   = relu(W1z^T z + c1 + te_t)        c1 = ctx @ W1[Z:] + b1 (host)
    u'    = dt * W2^T hid
    z'    = a_t z + u' + eps'_t              a_t = 1 + beta_f dt,
                                             eps' = bf16(s_t eps), s=sqrt(2 b dt) s0
    logw += 0.5 (eps'/s_t)^2 - k_t (z - c_t z')^2 + log(s_t/sb_t)
            c_t = 1 - beta_b dt, sb = sqrt(2 beta_b dt) s0, k = 0.5/sb^2
  terminal: logw += -0.5 (z_T - mu)^2 + 0.5 eps0^2 + log(sig0)
  output = sum_z mean_b logw

Device computes ONLY the trajectory-dependent sums per (p, t, halfchunk):
  slots[p, 2t+j]  = sum_{b in chunk j} (z_prev - c_t z_next)^2
  slots[p, 64+j]  = sum_{b in chunk j} (z_T - mu)^2
Host adds the noise/constant terms and the k_t weighting.

Device layout: feature-major [Z=128 partitions, batch free], batch sharded
8 ways (BS=2048/core, NP=2 chunks of 1024). Matmuls fp8 DoubleRow (K=256):
  hp  = [W1z; I]^T [z; c1]     (c1-add fused via identity rows)
  zps = a_t z (scaled-ident bf16 MM, variant "pe") + dt*W2^T hs + I^T eps'
W1 matmuls are emitted h-major across the two batch chunks so each DR weight
is loaded once per step.  State z is bf16; zc1 fp8 feeds the next W1.
"""
import math
import numpy as np

B, Z, H, HID, T = 16384, 128, 512, 512, 32
NCORES = 8
BS = B // NCORES          # 2048 batch rows per core
NP = 2                    # batch chunks of 1024 per core
CH = BS // NP             # 1024

_cache: dict = {}
# tuning knobs
KNOB = dict(
    variant="stt",               # pe: z' accumulated in PSUM via a_t-ident MM
                                 # stt: z' via DVE STT (baseline-style)
    relu_dve={(0, 3), (1, 3)},   # (np, h) relu chunks on DVE; rest ACT
    zc1_eng="act",               # fp8 z for next W1: act | dve   (variant pe)
    zc1_halves=True,             # emit zc1 evac as 2x512 halves
    zout_eng="dve",              # bf16 state evac (variant pe): dve | act
    cast_eng="pool+act",             # variant stt: fp8 cast engine pool|act|dve
    slots="sqdiff",              # sqdiff: fused custom-DVE sum((zin-c*zout)^2)
                                 # wvsq: w=TS, v=TT, sq per sq_eng
    w_eng="dve",                 # w = -c*zout:  pool | dve
    v_eng="dve",                 # v = w + zin:  dve | pool
    sq_eng="act",                # act (Square accum) | dve (TTR custom)
    hmajor=False,                # W1 MMs h-major across chunks (shared LDW)
    eps_bufs=4, hs_bufs=3, psh_bufs=2, psz_bufs=2,
    hi_zc1=True,                 # high_priority on zc1/cast evac
)


def _install_sqdiff():
    """Register a fused custom-DVE op: out = (in0*c0 + in1)^2, accum = sum.

    Computes sum_b (zin - c*zout)^2 in one DVE pass (replaces TS+TT+Square).
    The uop tables are written per-NEFF, so registration here is all that's
    needed; the sha pin is derived by a trial compile.
    """
    import operator
    import re

    from concourse import dve_ops
    from concourse.dve_spec import C0, Spec, Src0, Src1, Zero, sq

    for op in dve_ops.OPS:
        if op.name == "SQDIFF_ACC_ANT":
            return op
    def _ref(in0, in1, c0, c1, c2):
        b = (in0.astype(np.float32) * c0 + in1.astype(np.float32)) ** 2
        return b, b.reshape(b.shape[0], -1).sum(axis=-1, keepdims=True)

    spec = Spec(body=sq(Src0 * C0 + Src1), accum=operator.add,
                accum_init=Zero, reference=_ref)
    op = dve_ops.DveOp("SQDIFF_ACC_ANT", spec, subdim=False, uops_sha={})
    dve_ops._SUB_OPCODE_FOR_NAME[op.name] = (
        max(dve_ops._SUB_OPCODE_FOR_NAME.values()) + 1)
    for ver in ("v3", "v4"):
        try:
            dve_ops._COMPILE_CACHE.pop((op.name, ver), None)
            op.compile(ver)
        except ValueError as e:
            m = re.search(r'\["{}"\]="([0-9a-f]+)"'.format(ver), str(e))
            assert m, f"could not pin sha for {ver}: {e}"
            op.uops_sha[ver] = m.group(1)
            dve_ops._COMPILE_CACHE.pop((op.name, ver), None)
    dve_ops.OPS.append(op)
    dve_ops.CUSTOM_DVE_SPECS[op.name] = spec
    return op


def _default_consts():
    """c_t from the deterministic cosine beta schedule (matches setup_inputs)."""
    ts = np.linspace(1.0, 0.0, T)
    beta = (1.0 - 0.1) * np.cos(math.pi * (1.0 - ts) * 0.5) ** 2 + 0.1
    c_t = 1.0 - np.roll(beta, 1) / T
    return dict(c_t=[float(x) for x in c_t])


def _build_module(nop=False, reps=1, consts=None):
    import concourse.tile as tile
    from concourse import bacc, mybir

    if consts is None:
        consts = _default_consts()

    global _SQDIFF_OP
    if KNOB["slots"] == "sqdiff":
        _SQDIFF_OP = _install_sqdiff()

    f32 = mybir.dt.float32
    bf16 = mybir.dt.bfloat16
    f8e4 = mybir.dt.float8e4
    f8e5 = mybir.dt.float8e5
    AF = mybir.ActivationFunctionType
    ALU = mybir.AluOpType

    nc = bacc.Bacc("TRN2", target_bir_lowering=False, debug=False,
                   num_devices=NCORES)

    epsd = nc.dram_tensor("epsd", [T, 128, BS], bf16, kind="ExternalInput").ap()
    z0bd = nc.dram_tensor("z0bd", [128, BS], bf16, kind="ExternalInput").ap()
    z0f8d = nc.dram_tensor("z0f8d", [128, BS], f8e4, kind="ExternalInput").ap()
    mubd = nc.dram_tensor("mubd", [128, BS], bf16, kind="ExternalInput").ap()
    c1f8d = nc.dram_tensor("c1f8d", [4, 128, BS], f8e4, kind="ExternalInput").ap()
    w1drd = nc.dram_tensor("w1drd", [128, 4, 2, 128], f8e5, kind="ExternalInput").ap()
    w2drd = nc.dram_tensor("w2drd", [128, 2, 2, 128], f8e5, kind="ExternalInput").ap()
    identbd = nc.dram_tensor("identbd", [128, 128], bf16, kind="ExternalInput").ap()
    aidd = nc.dram_tensor("aidd", [128, T, 128], bf16, kind="ExternalInput").ap()
    tbld = nc.dram_tensor("tbld", [128, 64], f32, kind="ExternalInput").ap()
    tetd = nc.dram_tensor("tetd", [128, 128], f32, kind="ExternalInput").ap()
    outd = nc.dram_tensor("outd", [128, 2 * T + NP], f32, kind="ExternalOutput").ap()

    with tile.TileContext(nc) as tc:
        with (
            tc.tile_pool(name="const", bufs=1) as cpool,
            tc.tile_pool(name="state", bufs=1) as spool,
            tc.tile_pool(name="eps", bufs=KNOB["eps_bufs"]) as epool,
            tc.tile_pool(name="hs", bufs=KNOB["hs_bufs"]) as hpool,
            tc.tile_pool(name="scr", bufs=2) as scrp,
            tc.tile_pool(name="psH", bufs=KNOB["psh_bufs"], space="PSUM") as psH,
            tc.tile_pool(name="psZ", bufs=KNOB["psz_bufs"], space="PSUM") as psZ,
        ):
            if nop:
                out2 = spool.tile([128, 2 * T + NP], f32, tag="out2")
                nc.gpsimd.memset(out2[:], 0.0)
                nc.sync.dma_start(outd, out2[:])
            elif reps == 1:
                _emit(nc, tc, cpool, spool, epool, hpool, scrp, psH, psZ,
                      f32, bf16, f8e4, f8e5, AF, ALU,
                      epsd, z0bd, z0f8d, mubd, c1f8d, w1drd, w2drd, identbd,
                      aidd, tbld, tetd, outd, consts)
            else:
                with tc.For_i(0, reps, 1):
                    _emit(nc, tc, cpool, spool, epool, hpool, scrp, psH, psZ,
                          f32, bf16, f8e4, f8e5, AF, ALU,
                          epsd, z0bd, z0f8d, mubd, c1f8d, w1drd, w2drd,
                          identbd, aidd, tbld, tetd, outd, consts)

    nc.compile()
    return nc


def _emit(nc, tc, cpool, spool, epool, hpool, scrp, psH, psZ,
          f32, bf16, f8e4, f8e5, AF, ALU,
          epsd, z0bd, z0f8d, mubd, c1f8d, w1drd, w2drd, identbd, aidd, tbld,
          tetd, outd, consts):
    from concourse import mybir
    DR = mybir.MatmulPerfMode.DoubleRow
    c_t = consts["c_t"]          # python floats, len T
    variant = KNOB["variant"]

    # ---- resident constants ----
    w1dr = cpool.tile([128, 4, 2, 128], f8e5, tag="w1dr")
    nc.sync.dma_start(w1dr[:], w1drd)
    w2dr = cpool.tile([128, 2, 2, 128], f8e5, tag="w2dr")
    nc.sync.dma_start(w2dr[:], w2drd)
    identb = cpool.tile([128, 128], bf16, tag="identb")
    nc.sync.dma_start(identb[:], identbd)
    tbl = cpool.tile([128, 64], f32, tag="tbl")
    nc.sync.dma_start(tbl[:], tbld)
    tet = cpool.tile([128, 128], f32, tag="tet")
    nc.sync.dma_start(tet[:], tetd)
    if variant == "pe":
        aid = cpool.tile([128, T, 128], bf16, tag="aid")
        nc.sync.dma_start(aid[:], aidd)

    # ---- state ----
    zc1 = spool.tile([128, 5, BS], f8e4, tag="zc1")
    nc.sync.dma_start(zc1[:, 0, :], z0f8d)
    nc.sync.dma_start(zc1[:, 1:5, :], c1f8d.rearrange("h p b -> p h b"))
    zA = spool.tile([128, BS], bf16, tag="zA")
    zB = spool.tile([128, BS], bf16, tag="zB")
    nc.sync.dma_start(zA[:], z0bd)
    mub = spool.tile([128, BS], bf16, tag="mub")
    nc.sync.dma_start(mub[:], mubd)
    slots = spool.tile([128, 2 * T + NP], f32, tag="slots")
    zbuf = [zA, zB]

    def relu_one(np_, h, hs, hp, t):
        tecol = tet[:, h * 32 + t: h * 32 + t + 1]
        if (np_, h) in KNOB["relu_dve"]:
            nc.vector.tensor_scalar(hs[:, h, :], hp[:], scalar1=tecol,
                                    scalar2=0.0, op0=ALU.add, op1=ALU.max)
        else:
            nc.scalar.activation(hs[:, h, :], hp[:], AF.Relu,
                                 bias=tecol, scale=1.0)

    # ---- main loop (fully unrolled) ----
    for t in range(T):
        zin = zbuf[t % 2]
        zout = zbuf[(t + 1) % 2]
        acol = tbl[:, t:t + 1]
        ept = epool.tile([128, BS], bf16, tag="eps")
        nc.sync.dma_start(ept[:], epsd[t])

        hss = []
        for np_ in range(NP):
            hs = hpool.tile([128, 4, CH], f8e4, tag=f"hs{np_}")
            hss.append(hs)

        # --- W1 + relu ---
        def w1_pair(np_, h):
            base = np_ * CH
            hp = psH.tile([128, CH], f32, tag="hp")
            rhs0 = zc1[:, 0:h + 2:h + 1, base:base + 512]
            rhs1 = zc1[:, 0:h + 2:h + 1, base + 512:base + CH]
            nc.tensor.matmul(hp[:, 0:512], lhsT=w1dr[:, h], rhs=rhs0,
                             start=True, stop=True, perf_mode=DR)
            nc.tensor.matmul(hp[:, 512:CH], lhsT=w1dr[:, h], rhs=rhs1,
                             start=True, stop=True, perf_mode=DR)
            relu_one(np_, h, hss[np_], hp, t)

        if KNOB["hmajor"]:
            for h in range(4):
                for np_ in range(NP):
                    w1_pair(np_, h)
        else:
            for np_ in range(NP):
                for h in range(4):
                    w1_pair(np_, h)

        # --- zps + state + slots per chunk ---
        for np_ in range(NP):
            base = np_ * CH
            nsl2 = slice(base, base + CH)
            hs = hss[np_]
            zps = psZ.tile([128, CH], f32, tag="zps")
            halves = [(slice(hh * 512, (hh + 1) * 512),
                       slice(base + hh * 512, base + (hh + 1) * 512))
                      for hh in range(2)]
            if variant == "pe":
                for osl, zsl in halves:
                    nc.tensor.matmul(zps[:, osl], lhsT=aid[:, t],
                                     rhs=zin[:, zsl], start=True, stop=False)
                for osl, zsl in halves:
                    nc.tensor.matmul(zps[:, osl], lhsT=identb[:],
                                     rhs=ept[:, zsl], start=False, stop=False)
                st0 = False
            else:
                for osl, zsl in halves:
                    nc.tensor.matmul(zps[:, osl], lhsT=identb[:],
                                     rhs=ept[:, zsl], start=True, stop=False)
                st0 = False
            for osl, _ in halves:
                nc.tensor.matmul(zps[:, osl], lhsT=w2dr[:, 0],
                                 rhs=hs[:, 0:2, osl], start=st0, stop=False,
                                 perf_mode=DR)
            for osl, _ in halves:
                nc.tensor.matmul(zps[:, osl], lhsT=w2dr[:, 1],
                                 rhs=hs[:, 2:4, osl], start=False, stop=True,
                                 perf_mode=DR)

            import contextlib
            hi = tc.high_priority() if KNOB["hi_zc1"] else contextlib.nullcontext()
            if variant == "pe":
                # zc1 fp8 = cast(zps) straight from PSUM (critical path)
                ev = ([(slice(0, 512), slice(base, base + 512)),
                       (slice(512, CH), slice(base + 512, base + CH))]
                      if KNOB["zc1_halves"] else [(slice(0, CH), nsl2)])
                with hi:
                    for osl, zsl in ev:
                        if KNOB["zc1_eng"] == "act":
                            nc.scalar.copy(zc1[:, 0, zsl], zps[:, osl])
                        else:
                            nc.vector.tensor_copy(zc1[:, 0, zsl], zps[:, osl])
                # bf16 state evac (off critical path)
                if KNOB["zout_eng"] == "dve":
                    nc.vector.tensor_copy(zout[:, nsl2], zps[:])
                else:
                    nc.scalar.copy(zout[:, nsl2], zps[:])
            else:
                # z' = a_t z + zps on DVE (halves), then fp8 cast
                with hi:
                    for osl, zsl in halves:
                        nc.vector.scalar_tensor_tensor(
                            zout[:, zsl], in0=zin[:, zsl], scalar=acol,
                            in1=zps[:, osl], op0=ALU.mult, op1=ALU.add)
                    for hh, (osl, zsl) in enumerate(halves):
                        eng = KNOB["cast_eng"]
                        if eng == "pool+act":
                            eng = "pool" if hh == 0 else "act"
                        if eng == "pool":
                            nc.gpsimd.tensor_copy(zc1[:, 0, zsl], zout[:, zsl])
                        elif eng == "act":
                            nc.scalar.copy(zc1[:, 0, zsl], zout[:, zsl])
                        else:
                            nc.vector.tensor_copy(zc1[:, 0, zsl], zout[:, zsl])

            # --- slots: sum_b (zin - c*zout)^2 ---
            col = slots[:, 2 * t + np_: 2 * t + np_ + 1]
            if KNOB["slots"] == "sqdiff":
                so = scrp.tile([128, CH], bf16, tag=f"scrSo{np_}")
                nc.vector._custom_dve(
                    _SQDIFF_OP, out=so[:], in0=zout[:, nsl2],
                    in1=zin[:, nsl2], s0=float(-c_t[t]), accum_out=col)
                continue
            w = scrp.tile([128, CH], bf16, tag=f"scrW{np_}")
            if KNOB["w_eng"] == "pool":
                nc.gpsimd.tensor_scalar(w[:], zout[:, nsl2],
                                        scalar1=float(-c_t[t]), scalar2=None,
                                        op0=ALU.mult)
            else:
                nc.vector.tensor_scalar(w[:], zout[:, nsl2],
                                        scalar1=float(-c_t[t]), scalar2=None,
                                        op0=ALU.mult)
            v = scrp.tile([128, CH], bf16, tag=f"scrV{np_}")
            if KNOB["v_eng"] == "pool":
                nc.gpsimd.tensor_tensor(v[:], w[:], zin[:, nsl2], op=ALU.add)
            else:
                nc.vector.tensor_add(v[:], w[:], zin[:, nsl2])
            if KNOB["sq_eng"] == "act":
                sd = scrp.tile([128, 1], f32, tag=f"scrA{np_}")
                nc.scalar.activation(sd[:].broadcast_to((128, CH)), v[:],
                                     AF.Square, bias=0.0, scale=1.0,
                                     accum_out=col)
            else:
                so = scrp.tile([128, CH], bf16, tag=f"scrSo{np_}")
                nc.vector.tensor_tensor_reduce(
                    out=so[:], in0=v[:], in1=v[:], scale=1.0, scalar=0.0,
                    op0=ALU.mult, op1=ALU.add, accum_out=col)

    # ---- terminal: slots[64+np] = sum_b (z_T - mu)^2 ----
    zfin = zbuf[T % 2]
    for np_ in range(NP):
        nsl2 = slice(np_ * CH, (np_ + 1) * CH)
        vT = scrp.tile([128, CH], bf16, tag=f"scrV{np_}")
        nc.vector.tensor_sub(vT[:], zfin[:, nsl2], mub[:, nsl2])
        sT = scrp.tile([128, 1], f32, tag=f"scrA{np_}")
        nc.vector.scalar_tensor_tensor(
            sT[:].broadcast_to((128, CH)), in0=vT[:], scalar=1.0, in1=vT[:],
            op0=ALU.mult, op1=ALU.mult,
            accum_out=slots[:, 2 * T + np_: 2 * T + np_ + 1])

    nc.sync.dma_start(outd, slots[:])


def _host_prep(inputs):
    """Numpy-only preprocessing: dtype casts, transposes, shards, tables.

    Returns (in_maps, host_terms, consts).
    """
    import ml_dtypes
    bf16 = ml_dtypes.bfloat16
    f8e4 = ml_dtypes.float8_e4m3fn
    f8e5 = ml_dtypes.float8_e5m2

    ctx = np.asarray(inputs["context_embedding"], np.float32)
    eps0 = np.asarray(inputs["eps0"], np.float32)
    eps = np.asarray(inputs["eps"], np.float32)
    beta = np.asarray(inputs["beta_schedule"], np.float64)
    sig0 = float(np.asarray(inputs["sigma0"], np.float32)[0])
    W1 = np.asarray(inputs["W1"], np.float32)
    b1 = np.asarray(inputs["b1"], np.float32)
    W2 = np.asarray(inputs["W2"], np.float32)
    b2 = np.asarray(inputs["b2"], np.float32)
    te = np.asarray(inputs["t_emb"], np.float32)
    mu = np.asarray(inputs["target_mu"], np.float32)

    dt = 1.0 / T
    bb = np.roll(beta, 1)
    a_t = 1.0 + beta * dt
    c_t = 1.0 - bb * dt
    s_t = np.sqrt(2.0 * beta * dt) * sig0
    sb_t = np.sqrt(2.0 * bb * dt) * sig0
    k_t = 0.5 / sb_t ** 2
    const_per = float(np.sum(np.log(s_t) - np.log(sb_t)))

    if np.any(b2):
        raise NotImplementedError("nonzero b2 not supported by this kernel")

    # pre-scaled noise eps' = bf16(s_t * eps), transposed to [T, Z, B]
    epsb = (eps * s_t[:, None, None].astype(np.float32)).astype(bf16)
    host_eps = 0.0
    for t in range(T):
        host_eps += 0.5 * float(
            (epsb[t].astype(np.float32) ** 2).sum(dtype=np.float64)
        ) / float(s_t[t] ** 2)
    epsb_T = np.ascontiguousarray(epsb.transpose(0, 2, 1))  # [T, Z, B]

    z0b = (np.float32(sig0) * eps0).astype(bf16)            # [B, Z]
    host_e0 = 0.5 * float(
        (z0b.astype(np.float32) ** 2).sum(dtype=np.float64)
    ) / (sig0 ** 2)
    z0b_T = np.ascontiguousarray(z0b.T)                     # [Z, B]
    z0f8_T = z0b_T.astype(f8e4)
    mub_T = np.ascontiguousarray(mu.T.astype(bf16))         # [Z, B]

    c1 = (ctx @ W1[Z:] + b1).astype(np.float32)             # [B, HID]
    c1f8 = np.ascontiguousarray(c1.T).reshape(4, 128, B).astype(f8e4)

    # W1-DoubleRow fused weights: [ki, h, ko, m]; ko=0 -> W1z, ko=1 -> I
    w1dr = np.zeros((128, 4, 2, 128), np.float32)
    w1z = W1[:Z]                                            # [128, 512]
    idx = np.arange(128)
    for h in range(4):
        w1dr[:, h, 0, :] = w1z[:, h * 128:(h + 1) * 128]
        w1dr[idx, h, 1, idx] = 1.0
    w1dr = w1dr.astype(f8e5)

    # W2-DoubleRow weights: [ki, pair, ko, m] = dt * W2[(2p+ko)*128+ki, m]
    w2s = (W2 * np.float32(dt)).reshape(2, 2, 128, 128)     # [p, ko, ki, m]
    w2dr = np.ascontiguousarray(w2s.transpose(2, 0, 1, 3)).astype(f8e5)

    identb = np.eye(128, dtype=bf16)

    # per-step scaled identities a_t * I for the z-state matmul, [ki, t, m]
    aidd = np.zeros((128, T, 128), np.float32)
    aidd[idx, :, idx] = a_t[None, :].astype(np.float32)
    aidd = aidd.astype(bf16)

    tbl = np.zeros((128, 64), np.float32)
    tbl[:, 0:T] = a_t[None, :].astype(np.float32)
    tbl[:, 32:64] = -c_t[None, :].astype(np.float32)

    tet = np.zeros((128, 128), np.float32)
    for h in range(4):
        tet[:, h * 32:(h + 1) * 32] = te[:, h * 128:(h + 1) * 128].T

    in_maps = []
    for c in range(NCORES):
        bs = slice(c * BS, (c + 1) * BS)
        in_maps.append({
            "epsd": np.ascontiguousarray(epsb_T[:, :, bs]),
            "z0bd": np.ascontiguousarray(z0b_T[:, bs]),
            "z0f8d": np.ascontiguousarray(z0f8_T[:, bs]),
            "mubd": np.ascontiguousarray(mub_T[:, bs]),
            "c1f8d": np.ascontiguousarray(c1f8[:, :, bs]),
            "w1drd": w1dr,
            "w2drd": w2dr,
            "identbd": identb,
            "aidd": aidd,
            "tbld": tbl,
            "tetd": tet,
        })
    host_terms = dict(
        host_sum=host_eps + host_e0
        + B * Z * (const_per + math.log(sig0)),
        k_t=k_t,
    )
    consts = dict(c_t=[float(x) for x in c_t])
    return in_maps, host_terms, consts


def _assemble(results, host_terms):
    """Combine per-core slot outputs with the host terms."""
    k_t = host_terms["k_t"]
    dev = 0.0
    for res in results:
        o = res["outd"].astype(np.float64)                  # [128, 66]
        st = o[:, 0:2 * T].reshape(128, T, NP).sum(axis=2)  # [128, T]
        dev += float((st * k_t[None, :]).sum()) + 0.5 * float(
            o[:, 2 * T:].sum())
    total = (host_terms["host_sum"] - dev) / B
    return np.float32(total)


def _install_neff_cache():
    """Cache walrus NEFF output by BIR hash (compile takes minutes otherwise)."""
    import hashlib
    import os
    import shutil

    from concourse import bass2jax

    if getattr(bass2jax, "_ant_neff_cache_installed", False):
        return
    orig = bass2jax.compile_bir_kernel
    cache_dir = os.environ.get("BASS_NEFF_CACHE", "/tmp/neff_cache")

    def cached(bir_json, tmpdir, neff_name="file.neff"):
        os.makedirs(cache_dir, exist_ok=True)
        key = hashlib.sha256(bir_json if isinstance(bir_json, bytes)
                             else bir_json.encode()).hexdigest()[:24]
        hit = os.path.join(cache_dir, f"{key}.neff")
        dst = os.path.join(tmpdir, neff_name)
        if os.path.exists(hit):
            shutil.copy(hit, dst)
            return dst
        out = orig(bir_json, tmpdir, neff_name)
        shutil.copy(out, hit)
        return out

    bass2jax.compile_bir_kernel = cached
    bass2jax._ant_neff_cache_installed = True


def kernel(**inputs) -> np.ndarray:
    from concourse import bass_utils

    _install_neff_cache()
    in_maps, host_terms, consts = _host_prep(inputs)
    key = ("nc", tuple(consts["c_t"]))
    if key not in _cache:
        _cache[key] = _build_module(consts=consts)
        _cache["nc"] = _cache[key]
    nc = _cache[key]

    res = bass_utils.run_bass_kernel_spmd(nc, in_maps, core_ids=list(range(NCORES)))
    _cache["last_res"] = res
    return _assemble(res.results, host_terms)
